# revision 15
# baseline (speedup 1.0000x reference)
"""Trainium2 Bass kernel for a dense causal-attention transformer block.

Reference computation (fp32, B=2, S=2048, D=2048, H=16, HD=128):
    qkv = x @ Wqkv ; q,k,v split per head
    scores = (q @ k^T) * HD**-0.5, causal mask, softmax
    o = softmax(scores) @ v ; out = o @ Wo

Sharding: tensor-parallel over heads (4 groups of 4 heads) x data-parallel
over batch (2) = 8 cores. Each core computes a partial output projection
(its 512 o-channels x Wo rows); the host sums the 4 partials per batch.

Device layout tricks:
  - All big GEMMs run as fp8e4m3 DoubleRow matmuls (256-deep contraction
    per instruction, 0.5 PE cycles/row = 4x bf16 MAC throughput), with
    residual compensation for accuracy: operands are split hi+lo in fp8
    and three fp8 passes (hi*Whi + lo*Whi + hi16*Wlo) reproduce ~bf16
    accuracy at 0.75x the bf16 PE cost. All passes share one PSUM scale
    (16*x@W) via a power-of-16 scale ladder; the PSUM->SBUF copy unscales.
  - qT/kT are produced channels-on-partitions (bf16) so score tiles come
    out TRANSPOSED [keys=128, queries=512]; the softmax normalizer is a
    matmul with an all-ones fp8 lhsT (no cross-partition reduce).
  - exp emits fp8 attention weights directly (bias -SHIFT keeps the max
    under fp8e4m3's 240 limit); AV runs 2 fp8 DoubleRow passes
    (a @ (v_hi + v_lo)) and the normalizer sums the same fp8 a, so
    normalization cancels most of a's quantization error.
  - HD**-0.5 and all 1/16 unscales are folded into ACT/DVE copy scales.
"""

import numpy as np
import ml_dtypes

BF16 = ml_dtypes.bfloat16
E4 = ml_dtypes.float8_e4m3

B = 2
S = 2048
D = 2048
H = 16
HD = 128
P = 128
G = 4            # TP groups (heads per group = 4)
NH = H // G      # heads per core = 4
CH = NH * HD     # o-channels per core = 512
NJ = S // 512    # 4 S-chunks of 512
KK = D // P      # 16 contraction tiles
NP = KK // 2     # 8 DoubleRow k-tile pairs
ST = S // P      # 16 sequence row-tiles

SHIFT = 3.0      # exp(sc - SHIFT): keeps max a ~58 < fp8e4m3 max 240
S16 = 16.0       # residual scale ladder step

_progs = {}


def _build(repeat=1):
    """Build (once) the single-core Bass/Tile program shared by all 8 cores.

    repeat>1 executes the whole computation that many times inside one NEFF
    (used only for overhead-free timing via T(xN)-T(x1) differencing).
    """
    key = repeat
    if key in _progs:
        return _progs[key]

    import concourse.tile as tile
    from concourse import bacc, mybir

    f32 = mybir.dt.float32
    bf16 = mybir.dt.bfloat16
    fp8 = mybir.dt.float8e4

    nc = bacc.Bacc("TRN2", target_bir_lowering=False, debug=False)

    # DRAM I/O, pre-packed on host so every DMA is contiguous per partition.
    # x{hi,lo,hs}: [p, nj, kk, q] = xT chunk layout (x[b].T tiled), fp8:
    #   hi = fp8(x), lo = fp8(x - hi), hs = fp8(x/16)
    # wq/wk {hi,lo}: [p, mi, kk, m] column-sharded Wqkv in the 16/256 ladder
    # wv {hi,lo}: [p, kk, n] (rhs layout)
    # wo {hi,lo}: [p, h, ncol, n] (row-sharded Wo)
    # masks: [k, j, q] fp8 binary causal masks for the diagonal positions
    # out: [p, si, col] partial output (bf16; host sums in fp32)
    xhi_d = nc.dram_tensor("xhi", (P, NJ, KK, 512), fp8, kind="ExternalInput")
    xlo_d = nc.dram_tensor("xlo", (P, NJ, KK, 512), fp8, kind="ExternalInput")
    xhs_d = nc.dram_tensor("xhs", (P, NJ, KK, 512), fp8, kind="ExternalInput")
    wqh_d = nc.dram_tensor("wqhi", (P, NH, KK, P), fp8, kind="ExternalInput")
    wql_d = nc.dram_tensor("wqlo", (P, NH, KK, P), fp8, kind="ExternalInput")
    wkh_d = nc.dram_tensor("wkhi", (P, NH, KK, P), fp8, kind="ExternalInput")
    wkl_d = nc.dram_tensor("wklo", (P, NH, KK, P), fp8, kind="ExternalInput")
    wvh_d = nc.dram_tensor("wvhi", (P, KK, CH), fp8, kind="ExternalInput")
    wvl_d = nc.dram_tensor("wvlo", (P, KK, CH), fp8, kind="ExternalInput")
    woh_d = nc.dram_tensor("wohi", (P, NH, NJ, 512), fp8, kind="ExternalInput")
    wol_d = nc.dram_tensor("wolo", (P, NH, NJ, 512), fp8, kind="ExternalInput")
    mask_d = nc.dram_tensor("masks", (P, NH, 512), fp8, kind="ExternalInput")
    out_d = nc.dram_tensor("out", (P, ST, D), bf16, kind="ExternalOutput")

    dram = (xhi_d, xlo_d, xhs_d, wqh_d, wql_d, wkh_d, wkl_d, wvh_d, wvl_d,
            woh_d, wol_d, mask_d, out_d)

    with tile.TileContext(nc) as tc:
        with (
            tc.tile_pool(name="persist", bufs=1) as pp,
            tc.tile_pool(name="psumA", bufs=2, space="PSUM") as psA,
            tc.tile_pool(name="psumW", bufs=2, space="PSUM") as psW,
            tc.tile_pool(name="psumB", bufs=1, space="PSUM") as psB,
        ):
            for rep in range(repeat):
                _emit_once(nc, tc, tile, mybir, pp, psA, psW, psB,
                           dram, f32, bf16, fp8, rep)

    nc.compile()
    _progs[key] = nc
    return nc


def _emit_once(nc, tc, tile, mybir, pp, psA, psW, psB, dram,
               f32, bf16, fp8, rep):
    (xhi_d, xlo_d, xhs_d, wqh_d, wql_d, wkh_d, wkl_d, wvh_d, wvl_d,
     woh_d, wol_d, mask_d, out_d) = dram
    r = f"r{rep}_"
    EXP = mybir.ActivationFunctionType.Exp
    COPY = mybir.ActivationFunctionType.Copy
    DR = mybir.MatmulPerfMode.DoubleRow
    SUB = mybir.AluOpType.subtract
    MULT = mybir.AluOpType.mult

    # per-head-group weight tiles so the first matmul group only depends on
    # a small DMA, not the whole weight
    wqh_t = [pp.tile([P, KK, P], fp8, name=f"{r}wqh{mi}", tag=f"wqh{mi}")
             for mi in range(NH)]
    wql_t = [pp.tile([P, KK, P], fp8, name=f"{r}wql{mi}", tag=f"wql{mi}")
             for mi in range(NH)]
    wkh_t = [pp.tile([P, KK, P], fp8, name=f"{r}wkh{mi}", tag=f"wkh{mi}")
             for mi in range(NH)]
    wkl_t = [pp.tile([P, KK, P], fp8, name=f"{r}wkl{mi}", tag=f"wkl{mi}")
             for mi in range(NH)]
    # wv (phase 1) and wo (phase 3) share slots
    wvh_sb = pp.tile([P, KK, CH], fp8, name=r + "wvh_sb", tag="wvwoH")
    wvl_sb = pp.tile([P, KK, CH], fp8, name=r + "wvl_sb", tag="wvwoL")
    q_sb = pp.tile([P, NH, S], bf16, name=r + "q_sb", tag="q")
    k_sb = pp.tile([P, NH, S], bf16, name=r + "k_sb", tag="k")
    vh_sb = pp.tile([P, ST, CH], fp8, name=r + "vh_sb", tag="vh")
    vl_sb = pp.tile([P, ST, CH], fp8, name=r + "vl_sb", tag="vl")
    oh_sb = pp.tile([P, NH, S], fp8, name=r + "oh_sb", tag="oh")
    ol_sb = pp.tile([P, NH, S], fp8, name=r + "ol_sb", tag="ol")
    os_sb = pp.tile([P, NH, S], fp8, name=r + "os_sb", tag="os")
    mask_sb = pp.tile([P, NH, 512], fp8, name=r + "mask_sb", tag="mask")
    ones_sb = pp.tile([P, 2, P], fp8, name=r + "ones_sb", tag="ones")
    ebias = pp.tile([P, 1], f32, name=r + "ebias", tag="ebias")

    nc.gpsimd.memset(ones_sb[:], 1.0)
    nc.gpsimd.memset(ebias[:], -SHIFT)

    QS = float(HD) ** -0.5 / S16   # q evac scale (HD^-.5 folded here)
    KS = 1.0 / S16

    # ---------------- Phase 1: QKV projections ----------------
    with (
        tc.tile_pool(name=r + "xhp", bufs=2) as xhp,
        tc.tile_pool(name=r + "xlp", bufs=2) as xlp,
        tc.tile_pool(name=r + "xsp", bufs=2) as xsp,
        tc.tile_pool(name=r + "vtp", bufs=2) as vtp,
    ):
        xhi_c = {}
        xlo_c = {}
        xhs2_c = {}
        # DMA spread over the 3 queues (SP-HWDGE, ACT-HWDGE, SWDGE via
        # Pool) so the first q group's inputs all land within ~6us:
        #  SP:    wq hi0/hi1, xh0 (quarters), wq lo0/lo1, wq hi2/3 lo2/3,
        #         wk hi, wv hi, masks, xh1-3
        #  ACT:   xl0, wk lo, wv lo (done early; ACT then free for evacs)
        #  SWDGE: xs0-3, xl1-3
        xc0 = xhp.tile([P, KK, 512], fp8, name=f"{r}xh0", tag="xh")
        xhi_c[0] = xc0
        nc.sync.dma_start(wqh_t[0][:], wqh_d[:, 0])
        nc.sync.dma_start(wqh_t[1][:], wqh_d[:, 1])
        xl0 = xlp.tile([P, KK, 512], fp8, name=f"{r}xl0", tag="xl")
        xlo_c[0] = xl0
        nc.scalar.dma_start(xl0[:], xlo_d[:, 0])
        xs0 = xsp.tile([P, KK, 512], fp8, name=f"{r}xs0", tag="xs")
        xhs2_c[0] = xs0
        nc.gpsimd.dma_start(xs0[:], xhs_d[:, 0])
        for qtr in range(4):
            nc.sync.dma_start(
                xc0[:, qtr * KK // 4:(qtr + 1) * KK // 4],
                xhi_d[:, 0, qtr * KK // 4:(qtr + 1) * KK // 4])
        nc.scalar.dma_start(wql_t[0][:], wql_d[:, 0])
        nc.scalar.dma_start(wql_t[1][:], wql_d[:, 1])
        for mi in (2, 3):
            nc.sync.dma_start(wqh_t[mi][:], wqh_d[:, mi])
            nc.sync.dma_start(wql_t[mi][:], wql_d[:, mi])
        for mi in range(NH):
            nc.sync.dma_start(wkh_t[mi][:], wkh_d[:, mi])
            nc.scalar.dma_start(wkl_t[mi][:], wkl_d[:, mi])
        nc.sync.dma_start(wvh_sb[:], wvh_d[:])
        nc.scalar.dma_start(wvl_sb[:], wvl_d[:])
        nc.sync.dma_start(mask_sb[:], mask_d[:])

        for nj in range(NJ):
            xh = xhi_c.get(nj)
            if xh is None:
                xh = xhp.tile([P, KK, 512], fp8, name=f"{r}xh{nj}", tag="xh")
                nc.sync.dma_start(xh[:], xhi_d[:, nj])
            xs = xhs2_c.get(nj)
            if xs is None:
                xs = xsp.tile([P, KK, 512], fp8, name=f"{r}xs{nj}", tag="xs")
                nc.gpsimd.dma_start(xs[:], xhs_d[:, nj])
            xl = xlo_c.get(nj)
            if xl is None:
                xl = xlp.tile([P, KK, 512], fp8, name=f"{r}xl{nj}", tag="xl")
                nc.gpsimd.dma_start(xl[:], xlo_d[:, nj])

            # qT, kT: [CH, S] channel-major. 3 fp8 DoubleRow passes per
            # 256-wide PSUM subtile; two head-slots share a 2-bank PSUM
            # tile so one ACT copy (with unscale) moves both out.
            for wh_t, wl_t, dst, dn, sc in ((wqh_t, wql_t, q_sb, "q", QS),
                                            (wkh_t, wkl_t, k_sb, "k", KS)):
                for mi0 in (0, 2):
                    pw = psW.tile([P, 1024], f32,
                                  name=f"{r}{dn}{nj}_{mi0}", tag="accW")
                    for half in (0, 1):
                        mi = mi0 + half
                        for sub in (0, 1):
                            out = pw[:, half * 512 + sub * 256:
                                     half * 512 + (sub + 1) * 256]
                            c0 = sub * 256
                            n_mm = 3 * NP
                            i = 0
                            for xp, wt in ((xh, wh_t[mi]), (xl, wh_t[mi]),
                                           (xs, wl_t[mi])):
                                for j in range(NP):
                                    nc.tensor.matmul(
                                        out, wt[:, 2 * j:2 * j + 2, :],
                                        xp[:, 2 * j:2 * j + 2, c0:c0 + 256],
                                        start=(i == 0), stop=(i == n_mm - 1),
                                        perf_mode=DR)
                                    i += 1
                    nc.scalar.activation(
                        dst[:, mi0:mi0 + 2, nj * 512:(nj + 1) * 512],
                        pw[:].rearrange("p (a b) -> p a b", a=2),
                        COPY, bias=0.0, scale=sc)
            # v: [S, CH] row-major (keys on partitions); x is stationary
            for si0 in (0, 2):
                pw = psW.tile([P, 1024], f32,
                              name=f"{r}v{nj}_{si0}", tag="accW")
                for half in (0, 1):
                    si = si0 + half
                    for sub in (0, 1):
                        out = pw[:, half * 512 + sub * 256:
                                 half * 512 + (sub + 1) * 256]
                        c0 = sub * 256
                        n_mm = 3 * NP
                        i = 0
                        for xp, wv in ((xh, wvh_sb), (xl, wvh_sb),
                                       (xs, wvl_sb)):
                            for j in range(NP):
                                nc.tensor.matmul(
                                    out,
                                    xp[:, 2 * j:2 * j + 2, si * P:(si + 1) * P],
                                    wv[:, 2 * j:2 * j + 2, c0:c0 + 256],
                                    start=(i == 0), stop=(i == n_mm - 1),
                                    perf_mode=DR)
                                i += 1
                st = 4 * nj + si0
                vhv = vh_sb[:, st:st + 2, :].rearrange("p a b -> p (a b)")
                vlv = vl_sb[:, st:st + 2, :].rearrange("p a b -> p (a b)")
                nc.scalar.activation(
                    vh_sb[:, st:st + 2, :],
                    pw[:].rearrange("p (a b) -> p a b", a=2),
                    COPY, bias=0.0, scale=KS)
                # v_lo = v - v_hi, with v = psum/16 rounded to bf16 first
                # (PSUM read must be on DVE: GPSIMD cannot access PSUM)
                vt = vtp.tile([P, 1024], bf16, name=f"{r}vt{nj}_{si0}",
                              tag="vt")
                nc.vector.tensor_scalar(vt[:], pw[:], KS, 0.0, MULT)
                nc.gpsimd.tensor_tensor(vlv, vt[:], vhv, SUB)

    # wo reuses wv's slots (Tile serializes the DMA after last wv read)
    woh_sb = pp.tile([P, NH, NJ, 512], fp8, name=r + "woh_sb", tag="wvwoH")
    wol_sb = pp.tile([P, NH, NJ, 512], fp8, name=r + "wol_sb", tag="wvwoL")
    nc.sync.dma_start(woh_sb[:], woh_d[:])
    nc.sync.dma_start(wol_sb[:], wol_d[:])

    # ---------- Phase 2+3: attention + output projection ----------
    with (
        tc.tile_pool(name=r + "apool", bufs=17) as apool,
        tc.tile_pool(name=r + "tpool", bufs=3) as tpool,
        tc.tile_pool(name=r + "rpool", bufs=2) as rpool,
        tc.tile_pool(name=r + "ostage", bufs=3) as ostage,
    ):
        def emit_A(qc, h):
            """scoresT [keys=128, queries=512] in bf16, exp to fp8.
            Two key tiles per 2-bank PSUM tile; pairs share the pair-min
            diagonal offset so DoubleRow consumers can use both slots."""
            qs, qe = qc * 512, (qc + 1) * 512
            ktmax = 4 * qc + 4
            pairs = []
            for kt0 in range(0, ktmax, 2):
                o0 = _diag_off(qc, kt0)
                pw = psW.tile([P, 1024], f32,
                              name=f"{r}st{qc}_{h}_{kt0}", tag="accW")
                for j2 in (0, 1):
                    kt = kt0 + j2
                    nc.tensor.matmul(
                        pw[:, j2 * 512 + o0:(j2 + 1) * 512],
                        k_sb[:, h, kt * P:(kt + 1) * P],
                        q_sb[:, h, qs + o0:qe], start=True, stop=True)
                a2 = apool.tile([P, 1024], fp8,
                                name=f"{r}a{qc}_{h}_{kt0}", tag="a")
                if o0 == 0:
                    nc.scalar.activation(a2[:], pw[:], EXP, bias=ebias[:])
                else:
                    nc.scalar.activation(
                        a2[:].rearrange("p (a b) -> p a b", a=2)[:, :, o0:],
                        pw[:].rearrange("p (a b) -> p a b", a=2)[:, :, o0:],
                        EXP, bias=ebias[:])
                for j2 in (0, 1):
                    kt = kt0 + j2
                    if kt >= 4 * qc:  # diagonal tile: causal 0/1 mask
                        sl = a2[:, j2 * 512 + o0:(j2 + 1) * 512]
                        nc.gpsimd.tensor_tensor(
                            sl, sl, mask_sb[:, kt - 4 * qc, o0:], MULT)
                pairs.append((kt0, a2, o0))
            return pairs

        def emit_B(qc, h, pairs):
            """AV (2 fp8 DoubleRow passes) + fp8 ones-matmul normalizer +
            divide + o hi/lo/hs production for one head."""
            qs, qe = qc * 512, (qc + 1) * 512
            po = psB.tile([P, 512], f32, name=f"{r}po{qc}_{h}", tag="po")
            pn = psB.tile([P, 512], f32, name=f"{r}pn{qc}_{h}", tag="pn")
            for hf in (0, 1):
                hq0 = hf * 256
                act = [(kt0, a2) for kt0, a2, o0 in pairs if o0 <= hq0]
                n_mm = 2 * len(act)
                i = 0
                for kt0, a2 in act:
                    rhs = a2[:].rearrange(
                        "p (a b) -> p a b", a=2)[:, :, hq0:hq0 + 256]
                    for vX in (vh_sb, vl_sb):
                        nc.tensor.matmul(
                            po[:, hq0:hq0 + 256],
                            vX[:, kt0:kt0 + 2, h * HD:(h + 1) * HD],
                            rhs, start=(i == 0), stop=(i == n_mm - 1),
                            perf_mode=DR)
                        i += 1
                for i, (kt0, a2) in enumerate(act):
                    rhs = a2[:].rearrange(
                        "p (a b) -> p a b", a=2)[:, :, hq0:hq0 + 256]
                    nc.tensor.matmul(
                        pn[:, hq0:hq0 + 256], ones_sb[:], rhs,
                        start=(i == 0), stop=(i == len(act) - 1),
                        perf_mode=DR)
            SUB = mybir.AluOpType.subtract
            MULT = mybir.AluOpType.mult
            rec = rpool.tile([P, 512], f32, name=f"{r}rc{qc}_{h}", tag="rec")
            nc.vector.reciprocal_approx_fast(rec[:], pn[:])
            t = tpool.tile([P, 512], bf16, name=f"{r}t{qc}_{h}", tag="t")
            nc.vector.tensor_tensor(t[:], po[:], rec[:], MULT)
            ohv = oh_sb[:, h, qs:qe]
            nc.gpsimd.tensor_copy(ohv, t[:])
            nc.gpsimd.tensor_tensor(ol_sb[:, h, qs:qe], t[:], ohv, SUB)
            nc.vector.tensor_scalar(os_sb[:, h, qs:qe], t[:],
                                    1.0 / S16, 0.0, MULT)

        def emit_proj(qc):
            """Output projection for one 512-query chunk: 3 fp8 DoubleRow
            passes over head-pairs, unscale 1/16 at the stage copy
            (alternating DVE/Pool to balance the vector engines)."""
            for si in range(4 * qc, 4 * qc + 4):
                for nc0 in (0, 2):
                    stg = ostage.tile([P, 1024], bf16,
                                      name=f"{r}os{si}_{nc0}", tag="ostg")
                    for half in (0, 1):
                        ncol = nc0 + half
                        acc = psA.tile([P, 512], f32,
                                       name=f"{r}pr{si}_{ncol}", tag="accA")
                        for sub in (0, 1):
                            out = acc[:, sub * 256:(sub + 1) * 256]
                            c0 = sub * 256
                            i = 0
                            for oX, wX in ((oh_sb, woh_sb), (ol_sb, woh_sb),
                                           (os_sb, wol_sb)):
                                for h0 in (0, 2):
                                    nc.tensor.matmul(
                                        out,
                                        oX[:, h0:h0 + 2, si * P:(si + 1) * P],
                                        wX[:, h0:h0 + 2, ncol, c0:c0 + 256],
                                        start=(i == 0), stop=(i == 5),
                                        perf_mode=DR)
                                    i += 1
                        nc.vector.tensor_scalar(
                            stg[:, half * 512:(half + 1) * 512], acc[:],
                            1.0 / S16, 0.0,
                            mybir.AluOpType.mult)
                    # alternate out-DMAs over both HWDGE queues so the
                    # final drain isn't serialized on SP
                    deng = nc.sync if (si + nc0) % 4 in (0, 3) else nc.scalar
                    deng.dma_start(
                        out_d[:, si, nc0 * 512:(nc0 + 2) * 512], stg[:])

        # software pipeline: at step t emit scores/exp for head-step t, the
        # AV/normalizer for step t-1 (its exps had a full step to finish),
        # and the projection for a chunk three steps after its last head —
        # the 3-step lag parks proj(NJ-2)'s PE work over the final exp wait
        steps = [(qc, h) for qc in range(NJ) for h in range(NH)]
        pend = None
        for t, (qc, h) in enumerate(steps):
            a = emit_A(qc, h)
            if pend is not None:
                emit_B(*pend)
            if t >= 4 and steps[t - 4][1] == NH - 1:
                emit_proj(steps[t - 4][0])
            pend = (qc, h, a)
        emit_B(*pend)
        emit_proj(NJ - 1)


def _diag_off(qc, kt):
    """First visible query column (within the 512 chunk) for key tile kt of
    chunk qc; 0 for fully-visible tiles."""
    if kt < 4 * qc:
        return 0
    return 128 * (kt - 4 * qc)


def _pack_inputs(x, Wqkv, Wo):
    """Host-side shard + pack into the per-core DMA-friendly layouts.
    Arrays are shared between cores where identical (x per batch, weights
    per TP group, masks global)."""
    masks = np.zeros((P, NH, 512), dtype=E4)
    k_idx = np.arange(P)[:, None]
    q_idx = np.arange(512)[None, :]
    for j in range(NH):
        masks[:, j, :] = (P * j + k_idx <= q_idx).astype(E4)

    def pack_x(a):       # [S, D] -> [p, nj, kk, q]
        return np.ascontiguousarray(
            a.reshape(NJ, 512, KK, P).transpose(3, 0, 2, 1))

    def pack_qk(a):      # [D, CH] -> [p, mi, kk, m]
        return np.ascontiguousarray(
            a.reshape(KK, P, NH, P).transpose(1, 2, 0, 3))

    def pack_v(a):       # [D, CH] -> [p, kk, n]
        return np.ascontiguousarray(
            a.reshape(KK, P, CH).transpose(1, 0, 2))

    def pack_o(a):       # [CH, D] -> [p, h, ncol, n]
        return np.ascontiguousarray(
            a.reshape(NH, P, NJ, 512).transpose(1, 0, 2, 3))

    def ladder(w):
        """W -> (fp8(16W), fp8(16*(16W - fp8(16W)))) scale ladder."""
        w = np.asarray(w, np.float32)
        hi = (S16 * w).astype(E4)
        lo = (S16 * (S16 * w - hi.astype(np.float32))).astype(E4)
        return hi, lo

    xps = []
    for b in range(B):
        xb = np.asarray(x[b], np.float32)
        hi = xb.astype(E4)
        lo = (xb - hi.astype(np.float32)).astype(E4)
        hs = (xb / S16).astype(E4)
        xps.append({"xhi": pack_x(hi), "xlo": pack_x(lo),
                    "xhs": pack_x(hs)})

    wmaps = []
    for g in range(G):
        wq = np.asarray(Wqkv[:, CH * g:CH * (g + 1)], np.float32)
        wk = np.asarray(Wqkv[:, D + CH * g:D + CH * (g + 1)], np.float32)
        wv = np.asarray(Wqkv[:, 2 * D + CH * g:2 * D + CH * (g + 1)],
                        np.float32)
        wo = np.asarray(Wo[CH * g:CH * (g + 1), :], np.float32)
        qh, ql = ladder(wq)
        kh, kl = ladder(wk)
        vh, vl = ladder(wv)
        oh, ol = ladder(wo)
        wmaps.append({
            "wqhi": pack_qk(qh), "wqlo": pack_qk(ql),
            "wkhi": pack_qk(kh), "wklo": pack_qk(kl),
            "wvhi": pack_v(vh), "wvlo": pack_v(vl),
            "wohi": pack_o(oh), "wolo": pack_o(ol),
        })

    return [{**xps[c // G], "masks": masks, **wmaps[c % G]}
            for c in range(8)]


def _unpack_outputs(results):
    """Sum the 4 TP partials per batch and restore [B, S, D]."""
    out = np.zeros((B, S, D), dtype=np.float32)
    for c, res in enumerate(results):
        b = c // G
        part = np.asarray(res["out"]).astype(np.float32)   # [p, si, col]
        out[b] += part.transpose(1, 0, 2).reshape(S, D)
    return out


def kernel(x, Wqkv, Wo, _trace=False, _trace_kwargs=None):
    from concourse import bass_utils

    nc = _build()
    in_maps = _pack_inputs(x, Wqkv, Wo)
    res = bass_utils.run_bass_kernel_spmd(
        nc, in_maps, core_ids=list(range(8)), trace=_trace,
        **(_trace_kwargs or {}))
    out = _unpack_outputs(res.results)
    if _trace:
        kernel.last_result = res
    return out


# revision 16
# speedup vs baseline: 2.0207x; 2.0207x over previous
"""Trainium2 Bass kernel for a dense causal-attention transformer block.

Reference computation (fp32, B=2, S=2048, D=2048, H=16, HD=128):
    qkv = x @ Wqkv ; q,k,v split per head
    scores = (q @ k^T) * HD**-0.5, causal mask, softmax
    o = softmax(scores) @ v ; out = o @ Wo

Sharding: tensor-parallel over heads (4 groups of 4 heads) x data-parallel
over batch (2) = 8 cores. Each core computes a partial output projection
(its 512 o-channels x Wo rows); the host sums the 4 partials per batch.

Device layout tricks:
  - All matmul inputs are bf16 (fp8/DoubleRow measured NO per-row win on
    this silicon, so residual-fp8 multi-pass schemes lose to plain bf16);
    PSUM accumulates fp32.
  - qT/kT are produced channels-on-partitions so score tiles come out
    TRANSPOSED [keys=128, queries=512]; softmax sum is then a matmul with
    an all-ones lhsT (no cross-partition reduce, no transposes anywhere).
  - No max-subtraction in softmax: scores ~ N(0,1), exp is safe in fp32,
    and masked entries are multiplied by 0 after exp.
  - HD**-0.5 scaling folded into Wq on the host.
  - Normalizer uses a quad-tree: Pool/DVE pre-add groups of 4 score
    tiles so the PE ones-matmul count drops ~4x for full tiles.
  - Input DMAs are spread over SP-HWDGE, ACT-HWDGE and the SWDGE queue
    so the first QKV group's operands land within ~6us; output DMAs
    alternate SP/ACT so the final drain isn't serialized.
"""

import numpy as np
import ml_dtypes

BF16 = ml_dtypes.bfloat16

B = 2
S = 2048
D = 2048
H = 16
HD = 128
P = 128
G = 4            # TP groups (heads per group = 4)
NH = H // G      # heads per core = 4
CH = NH * HD     # o-channels per core = 512
NJ = S // 512    # 4 S-chunks of 512
KK = D // P      # 16 contraction tiles
ST = S // P      # 16 sequence row-tiles

_progs = {}


def _build(repeat=1):
    """Build (once) the single-core Bass/Tile program shared by all 8 cores.

    repeat>1 executes the whole computation that many times inside one NEFF
    (used only for overhead-free timing via T(xN)-T(x1) differencing).
    """
    key = repeat
    if key in _progs:
        return _progs[key]

    import concourse.tile as tile
    from concourse import bacc, mybir

    f32 = mybir.dt.float32
    bf16 = mybir.dt.bfloat16
    EXP = mybir.ActivationFunctionType.Exp

    nc = bacc.Bacc("TRN2", target_bir_lowering=False, debug=False)

    # DRAM I/O, pre-packed on host so every DMA is contiguous per partition.
    # x:  [p, nj, kk, q]  = xT chunk layout (x[b].T tiled)
    # wq/wk: [p, mi, kk, m] (column-sharded Wqkv, q part prescaled by HD^-.5)
    # wv: [p, kk, n]      (rhs layout)
    # wo: [p, h, ncol, n] (row-sharded Wo)
    # masks: [k, j, q]    binary causal masks for the 4 diagonal positions
    # out: [p, si, col]   partial output (bf16; host sums in fp32)
    x_d = nc.dram_tensor("x", (P, NJ, KK, 512), bf16, kind="ExternalInput")
    wq_d = nc.dram_tensor("wq", (P, NH, KK, P), bf16, kind="ExternalInput")
    wk_d = nc.dram_tensor("wk", (P, NH, KK, P), bf16, kind="ExternalInput")
    wv_d = nc.dram_tensor("wv", (P, KK, CH), bf16, kind="ExternalInput")
    wo_d = nc.dram_tensor("wo", (P, NH, NJ, 512), bf16, kind="ExternalInput")
    mask_d = nc.dram_tensor("masks", (P, NH, 512), bf16, kind="ExternalInput")
    out_d = nc.dram_tensor("out", (P, ST, D), bf16, kind="ExternalOutput")

    with tile.TileContext(nc) as tc:
        with (
            tc.tile_pool(name="persist", bufs=1) as pp,
            tc.tile_pool(name="psumA", bufs=2, space="PSUM") as psA,
            tc.tile_pool(name="psumW", bufs=2, space="PSUM") as psW,
            tc.tile_pool(name="psumB", bufs=1, space="PSUM") as psB,
        ):
            for rep in range(repeat):
                _emit_once(nc, tc, tile, mybir, pp, psA, psW, psB,
                           x_d, wq_d, wk_d, wv_d, wo_d, mask_d, out_d,
                           f32, bf16, EXP, rep)

    nc.compile()
    _progs[key] = nc
    return nc


def _emit_once(nc, tc, tile, mybir, pp, psA, psW, psB,
               x_d, wq_d, wk_d, wv_d, wo_d, mask_d, out_d,
               f32, bf16, EXP, rep):
    r = f"r{rep}_"
    ADD = mybir.AluOpType.add
    # wq/wk as 4 per-head-group tiles so the first matmul group only
    # depends on a small DMA, not the whole weight
    wq_t = [pp.tile([P, KK, P], bf16, name=f"{r}wq{mi}", tag=f"wq{mi}")
            for mi in range(NH)]
    wk_t = [pp.tile([P, KK, P], bf16, name=f"{r}wk{mi}", tag=f"wk{mi}")
            for mi in range(NH)]
    # wv (phase 1) and wo (phase 3) share one slot
    wv_sb = pp.tile([P, KK, CH], bf16, name=r + "wv_sb", tag="wvwo")
    q_sb = pp.tile([P, NH, S], bf16, name=r + "q_sb", tag="q")
    k_sb = pp.tile([P, NH, S], bf16, name=r + "k_sb", tag="k")
    v_sb = pp.tile([P, ST, CH], bf16, name=r + "v_sb", tag="v")
    o_sb = pp.tile([P, NH, S], bf16, name=r + "o_sb", tag="o")
    mask_sb = pp.tile([P, NH, 512], bf16, name=r + "mask_sb", tag="mask")
    ones_sb = pp.tile([P, P], bf16, name=r + "ones_sb", tag="ones")
    zbias = pp.tile([P, 1], f32, name=r + "zbias", tag="zbias")

    nc.gpsimd.memset(ones_sb[:], 1.0)
    nc.gpsimd.memset(zbias[:], 0.0)

    # ---------------- Phase 1: QKV projections ----------------
    with tc.tile_pool(name=r + "xpool", bufs=2) as xpool:
        xcs = {}
        # DMA spread: first q group (wq0, wq1, full xc0) lands within ~6us
        #  SP:   wq0, xc0 q0-1, wq2, wk1, wk3, wv-half, x2, masks
        #  ACT:  wq1, xc0 q2-3, wq3, wk0, wk2, wv-half, x3
        #  SWDGE(Pool): x1
        xcs[0] = xpool.tile([P, KK, 512], bf16, name=f"{r}xc0", tag="xc")
        xc0 = xcs[0]
        nc.sync.dma_start(wq_t[0][:], wq_d[:, 0])
        nc.scalar.dma_start(wq_t[1][:], wq_d[:, 1])
        for qtr in range(4):
            eng = nc.sync if qtr < 2 else nc.scalar
            eng.dma_start(
                xc0[:, qtr * KK // 4:(qtr + 1) * KK // 4],
                x_d[:, 0, qtr * KK // 4:(qtr + 1) * KK // 4])
        nc.sync.dma_start(wq_t[2][:], wq_d[:, 2])
        nc.scalar.dma_start(wq_t[3][:], wq_d[:, 3])
        xcs[1] = xpool.tile([P, KK, 512], bf16, name=f"{r}xc1", tag="xc")
        nc.gpsimd.dma_start(xcs[1][:], x_d[:, 1])
        for mi in range(NH):
            eng = nc.scalar if mi % 2 == 0 else nc.sync
            eng.dma_start(wk_t[mi][:], wk_d[:, mi])
        nc.sync.dma_start(wv_sb[:, :KK // 2], wv_d[:, :KK // 2])
        nc.scalar.dma_start(wv_sb[:, KK // 2:], wv_d[:, KK // 2:])
        nc.sync.dma_start(mask_sb[:], mask_d[:])

        for nj in range(NJ):
            xc = xcs.get(nj)
            if xc is None:
                xc = xpool.tile([P, KK, 512], bf16, name=f"{r}xc{nj}",
                                tag="xc")
                eng = nc.sync if nj == 2 else nc.scalar
                eng.dma_start(xc[:], x_d[:, nj])
            # qT, kT: [CH, S] channel-major (per head = 128 partitions).
            # Two 16-MM accumulation groups fill the two banks of one
            # 2-bank PSUM tile so a single ACT copy moves both out.
            for w_t, dst, dn in ((wq_t, q_sb, "q"), (wk_t, k_sb, "k")):
                for mi0 in (0, 2):
                    pw = psW.tile([P, 1024], f32,
                                  name=f"{r}{dn}{nj}_{mi0}", tag="accW")
                    for half in (0, 1):
                        mi = mi0 + half
                        for kk in range(KK):
                            nc.tensor.matmul(
                                pw[:, half * 512:(half + 1) * 512],
                                w_t[mi][:, kk, :], xc[:, kk, :],
                                start=(kk == 0), stop=(kk == KK - 1))
                    nc.scalar.copy(
                        out=dst[:, mi0:mi0 + 2, nj * 512:(nj + 1) * 512],
                        in_=pw[:].rearrange("p (a b) -> p a b", a=2))
            # v: [S, CH] row-major (keys on partitions), same pairing
            for si0 in (0, 2):
                pw = psW.tile([P, 1024], f32,
                              name=f"{r}v{nj}_{si0}", tag="accW")
                for half in (0, 1):
                    si = si0 + half
                    for kk in range(KK):
                        nc.tensor.matmul(
                            pw[:, half * 512:(half + 1) * 512],
                            xc[:, kk, si * P:(si + 1) * P],
                            wv_sb[:, kk, :],
                            start=(kk == 0), stop=(kk == KK - 1))
                nc.scalar.copy(
                    out=v_sb[:, 4 * nj + si0:4 * nj + si0 + 2, :],
                    in_=pw[:].rearrange("p (a b) -> p a b", a=2))

    # wo reuses wv's slot (Tile serializes the DMA after last wv read)
    wo_sb = pp.tile([P, NH, NJ, 512], bf16, name=r + "wo_sb", tag="wvwo")
    nc.sync.dma_start(wo_sb[:], wo_d[:])

    # ---------- Phase 2+3: attention + output projection ----------
    eng_toggle = [0]

    def veng():
        eng_toggle[0] += 1
        return nc.gpsimd if eng_toggle[0] % 3 else nc.vector

    with (
        tc.tile_pool(name=r + "apool", bufs=18) as apool,
        tc.tile_pool(name=r + "tpool", bufs=12) as tpool,
        tc.tile_pool(name=r + "rpool", bufs=3) as rpool,
        tc.tile_pool(name=r + "ostage", bufs=4) as ostage,
    ):
        def emit_A(qc, h):
            """scoresT [keys=128, queries=512], two key tiles per 2-bank
            PSUM tile so exp runs as one [128,1024] ACT op."""
            qs, qe = qc * 512, (qc + 1) * 512
            ktmax = 4 * qc + 4
            a_slices = []
            for kt0 in range(0, ktmax, 2):
                pw = psW.tile([P, 1024], f32,
                              name=f"{r}st{qc}_{h}_{kt0}", tag="accW")
                offs = (_diag_off(qc, kt0), _diag_off(qc, kt0 + 1))
                for j2 in (0, 1):
                    kt = kt0 + j2
                    # diagonal tiles: queries < 128j are fully masked —
                    # compute, exp, mask and consume only visible columns
                    off = offs[j2]
                    nc.tensor.matmul(
                        pw[:, j2 * 512 + off:(j2 + 1) * 512],
                        k_sb[:, h, kt * P:(kt + 1) * P],
                        q_sb[:, h, qs + off:qe], start=True, stop=True)
                a2 = apool.tile([P, 1024], bf16,
                                name=f"{r}a{qc}_{h}_{kt0}", tag="a")
                if offs == (0, 0):
                    nc.scalar.activation(a2[:], pw[:], EXP, bias=zbias[:])
                else:
                    for j2 in (0, 1):
                        off = offs[j2]
                        nc.scalar.activation(
                            a2[:, j2 * 512 + off:(j2 + 1) * 512],
                            pw[:, j2 * 512 + off:(j2 + 1) * 512],
                            EXP, bias=zbias[:])
                for j2 in (0, 1):
                    kt = kt0 + j2
                    sl = a2[:, j2 * 512:(j2 + 1) * 512]
                    if kt >= 4 * qc:  # diagonal tile: causal 0/1 mask
                        off = offs[j2]
                        nc.vector.tensor_mul(
                            out=sl[:, off:], in0=sl[:, off:],
                            in1=mask_sb[:, kt - 4 * qc, off:])
                    a_slices.append(sl)
            return a_slices

        def emit_B(qc, h, a_slices):
            """AV accumulation + quad-tree normalizer + divide."""
            qs, qe = qc * 512, (qc + 1) * 512
            ktmax = 4 * qc + 4
            po = psB.tile([P, 512], f32, name=f"{r}po{qc}_{h}", tag="po")
            for kt in range(ktmax):
                # same column restriction as the scores; kt=0 is always a
                # full-width write, so every po column is initialized by the
                # start=True matmul
                off = _diag_off(qc, kt)
                nc.tensor.matmul(
                    po[:, off:], v_sb[:, kt, h * HD:(h + 1) * HD],
                    a_slices[kt][:, off:],
                    start=(kt == 0), stop=(kt == ktmax - 1))
            # normalizer: column sums of a over all key tiles, replicated
            # to all partitions by the all-ones lhsT. Full (off-diagonal)
            # tiles are pre-summed in quads on Pool/DVE so the PE does one
            # ones-matmul per 4; diagonal tiles go in individually,
            # restricted to their visible columns. First entry is always
            # full-width so the start=True matmul initializes every column.
            full = [a_slices[kt] for kt in range(ktmax) if kt < 4 * qc]
            diag = [(kt, _diag_off(qc, kt)) for kt in range(ktmax)
                    if kt >= 4 * qc]
            sum_rhs = []
            i = 0
            while i + 4 <= len(full):
                t1 = tpool.tile([P, 512], bf16,
                                name=f"{r}ts{qc}_{h}_{i}a", tag="tsum")
                t2 = tpool.tile([P, 512], bf16,
                                name=f"{r}ts{qc}_{h}_{i}b", tag="tsum")
                t3 = tpool.tile([P, 512], bf16,
                                name=f"{r}ts{qc}_{h}_{i}c", tag="tsum")
                veng().tensor_tensor(t1[:], full[i], full[i + 1], ADD)
                veng().tensor_tensor(t2[:], full[i + 2], full[i + 3], ADD)
                veng().tensor_tensor(t3[:], t1[:], t2[:], ADD)
                sum_rhs.append((0, t3[:]))
                i += 4
            if i + 2 <= len(full):
                t1 = tpool.tile([P, 512], bf16,
                                name=f"{r}ts{qc}_{h}_{i}p", tag="tsum")
                veng().tensor_tensor(t1[:], full[i], full[i + 1], ADD)
                sum_rhs.append((0, t1[:]))
                i += 2
            if i < len(full):
                sum_rhs.append((0, full[i]))
            sum_rhs += [(off, a_slices[kt][:, off:]) for kt, off in diag]
            pn = psB.tile([P, 512], f32, name=f"{r}pn{qc}_{h}", tag="pn")
            for i, (off, t) in enumerate(sum_rhs):
                nc.tensor.matmul(pn[:, off:], ones_sb[:], t,
                                 start=(i == 0),
                                 stop=(i == len(sum_rhs) - 1))
            rec = rpool.tile([P, 512], f32, name=f"{r}rc{qc}_{h}",
                             tag="rec")
            nc.vector.reciprocal_approx_fast(rec[:], pn[:])
            nc.vector.tensor_mul(out=o_sb[:, h, qs:qe],
                                 in0=po[:], in1=rec[:])

        def emit_proj(qc):
            for si in range(4 * qc, 4 * qc + 4):
                for nc0 in (0, 2):
                    # two column-block groups share one stage tile
                    stg = ostage.tile([P, 1024], bf16,
                                      name=f"{r}os{si}_{nc0}", tag="os")
                    for half in (0, 1):
                        ncol = nc0 + half
                        acc = psA.tile([P, 512], f32,
                                       name=f"{r}pr{si}_{ncol}", tag="accA")
                        for hh in range(NH):
                            nc.tensor.matmul(
                                acc[:], o_sb[:, hh, si * P:(si + 1) * P],
                                wo_sb[:, hh, ncol, :],
                                start=(hh == 0), stop=(hh == NH - 1))
                        nc.vector.tensor_copy(
                            out=stg[:, half * 512:(half + 1) * 512],
                            in_=acc[:])
                    # alternate out-DMAs over both HWDGE queues so the
                    # final drain isn't serialized on SP
                    deng = nc.sync if (si + nc0) % 4 in (0, 3) else nc.scalar
                    deng.dma_start(
                        out_d[:, si, nc0 * 512:(nc0 + 2) * 512], stg[:])

        # software pipeline: at step t emit scores/exp for head-step t, the
        # AV/normalizer for step t-1 (its exps had a full step to finish),
        # and the projection for a chunk four steps after its last head —
        # the 4-step lag parks proj(NJ-2)'s PE work over the final exp wait
        steps = [(qc, h) for qc in range(NJ) for h in range(NH)]
        pend = None
        for t, (qc, h) in enumerate(steps):
            a = emit_A(qc, h)
            if pend is not None:
                emit_B(*pend)
            if t >= 4 and steps[t - 4][1] == NH - 1:
                emit_proj(steps[t - 4][0])
            pend = (qc, h, a)
        emit_B(*pend)
        emit_proj(NJ - 1)


def _diag_off(qc, kt):
    """First visible query column (within the 512 chunk) for key tile kt of
    chunk qc; 0 for fully-visible tiles."""
    if kt < 4 * qc:
        return 0
    return 128 * (kt - 4 * qc)


def _pack_inputs(x, Wqkv, Wo):
    """Host-side shard + pack into the per-core DMA-friendly layouts.
    Arrays are shared between cores where identical (x per batch, weights
    per TP group, masks global)."""
    scale = np.float32(HD) ** np.float32(-0.5)
    masks = np.zeros((P, NH, 512), dtype=BF16)
    k_idx = np.arange(P)[:, None]
    q_idx = np.arange(512)[None, :]
    for j in range(NH):
        masks[:, j, :] = (P * j + k_idx <= q_idx).astype(BF16)

    xps = []
    for b in range(B):
        xb = np.asarray(x[b], dtype=np.float32)
        # xT packed: [p, nj, kk, q] with xT[128*kk+p, 512*nj+q] = xb[q', d']
        xps.append(np.ascontiguousarray(
            xb.astype(BF16).reshape(NJ, 512, KK, P).transpose(3, 0, 2, 1)))

    wmaps = []
    for g in range(G):
        wq = (np.asarray(Wqkv[:, CH * g:CH * (g + 1)], np.float32) * scale)
        wk = np.asarray(Wqkv[:, D + CH * g:D + CH * (g + 1)], np.float32)
        wv = np.asarray(Wqkv[:, 2 * D + CH * g:2 * D + CH * (g + 1)],
                        np.float32)
        wo = np.asarray(Wo[CH * g:CH * (g + 1), :], np.float32)
        wmaps.append({
            "wq": np.ascontiguousarray(
                wq.astype(BF16).reshape(KK, P, NH, P).transpose(1, 2, 0, 3)),
            "wk": np.ascontiguousarray(
                wk.astype(BF16).reshape(KK, P, NH, P).transpose(1, 2, 0, 3)),
            "wv": np.ascontiguousarray(
                wv.astype(BF16).reshape(KK, P, CH).transpose(1, 0, 2)),
            "wo": np.ascontiguousarray(
                wo.astype(BF16).reshape(NH, P, NJ, 512).transpose(1, 0, 2, 3)),
        })

    return [{"x": xps[c // G], "masks": masks, **wmaps[c % G]}
            for c in range(8)]


def _unpack_outputs(results):
    """Sum the 4 TP partials per batch and restore [B, S, D]."""
    out = np.zeros((B, S, D), dtype=np.float32)
    for c, res in enumerate(results):
        b = c // G
        part = np.asarray(res["out"]).astype(np.float32)   # [p, si, col]
        out[b] += part.transpose(1, 0, 2).reshape(S, D)
    return out


def kernel(x, Wqkv, Wo, _trace=False, _trace_kwargs=None):
    from concourse import bass_utils

    nc = _build()
    in_maps = _pack_inputs(x, Wqkv, Wo)
    res = bass_utils.run_bass_kernel_spmd(
        nc, in_maps, core_ids=list(range(8)), trace=_trace,
        **(_trace_kwargs or {}))
    out = _unpack_outputs(res.results)
    if _trace:
        kernel.last_result = res
    return out
